# revision 21
# baseline (speedup 1.0000x reference)
"""Multi-head causal self-attention (B=2, S=2048, E=1024, H=16, D=64) on 8 TRN2
NeuronCores.

Sharding: core c owns batch b = c//4 and head-group g = c%4 (4 heads each).
Per core, transpose-free layout: QT/KT [d_local=256, S] (d on partitions),
V [S, d_local] (t on partitions), scoresT [t, s] blocks via lhsT=KT-block,
softmax is unnormalized exp (scores ~N(0,1) in f32), output projection is
row-parallel (each core computes a partial [S, E] with its 256 channels of
Wp; host sums 4 bf16 partials per batch in f32 and adds bp).

Schedule: QKV projection for s-block i is interleaved with attention for
s-block i-1..i (causal: attention block i only needs K/V t-blocks <= 4i+3),
so the ACT-heavy attention exps overlap the PE-dense projection matmuls.

Softmax denominators come free from V tiles augmented per head with 64 ones
columns ([1s | v_h] -> M=128 AV matmul; rows 0:64 of the AV PSUM tile hold
the column sums replicated 64x), so normalization is a full-lane DVE
reciprocal + multiply. The causal mask is a post-exp DVE multiply of the
diagonal P block by an upper-tri 0/1 bf16 mask (SBUF-only 4x-mode, both
head groups in one op); waves process their diagonal t-blocks FIRST so
the mask hop hides inside the wave and the wave end flushes on mask-free
blocks.

Head pairs run as two sequential waves per s-block (PSUM economy); heads
within a pair sit at partition bases 0/64 so their K=64 score matmuls
run at the PE's double-rate half-contraction mode. AV consumes PT three
iterations later. Each block's attention iterations carry fillers: the
NEXT block's QKV PSUM-tile groups and earlier blocks' projection units,
weighted toward the exp-paced late blocks (block 2 keeps half of
proj(1); block 3 absorbs the rest), so the PE never drains while the
ACT engine works through the softmax exps.

Head pipelining: weights are dt-major so wq/wk split into per-dt halves;
the first x s-block splits into two 4-e-block halves. DMA order is
by time-of-first-use (scalar ring: biases+mask consts, wq0, wk0, wq1, wk1;
sync ring: xt0a, xt0b, wv, xt1, wp, xt2, xt3), so the first QKV tile's
inputs land ~4us earlier than a monolithic load. Junk warm-up matmuls
bridge the preamble->data window to keep the HAM clock gate open. The
final block's p=1 normalize is chunk-interleaved with its projection
units, whose PSUM drains alternate ACT/DVE and whose output DMAs
alternate sync/scalar rings.

Weights/activations are bf16 (matmul inputs); accumulation f32 in PSUM;
softmax exp/normalization f32; partial outputs bf16.
"""

import numpy as np
import ml_dtypes

import concourse.bass as bass
import concourse.tile as tile
from concourse import bacc, mybir
from concourse import bass_utils

B, S, E, H, D = 2, 2048, 1024, 16, 64
NCORES = 8
HPC = 4                 # heads per core
EL = HPC * D            # 256 local channels
SBW = 512               # s-block width
NSB = S // SBW          # 4
TBW = 128               # t-block width
NTB = S // TBW          # 16
NEB = E // 128          # 8 e-blocks
SCALE = 1.0 / np.sqrt(D)
NWARM = 8
NWARM2 = 14

F32 = mybir.dt.float32
BF16 = mybir.dt.bfloat16

# wq pack layout (bf16): [wq dt-major 2048 | bv 256 | tri 128 | tri 128]
WQP = NEB * EL
PBV = WQP          # bv
PUT = WQP + EL
WQW = PUT + 256

_BUILT = None


def _emit(tc, nc, d):
    Exp = mybir.ActivationFunctionType.Exp
    Copy = mybir.ActivationFunctionType.Copy

    with (
        tc.tile_pool(name="const", bufs=1) as cst,
        tc.tile_pool(name="big", bufs=1) as big,
        tc.tile_pool(name="ptp", bufs=8) as ptp,
        tc.tile_pool(name="bcsp", bufs=2) as bcsp,
        tc.tile_pool(name="outp", bufs=4) as outp,
        tc.tile_pool(name="ps", bufs=2, space="PSUM") as psp,
    ):
        # ---- PE warm-up: junk matmuls from preamble end (gated only by a
        # fast gpsimd memset) keep the PE busy through the input DMA window
        # so the HAM clock gate unthrottles early ----
        warm_a = cst.tile([128, 512], BF16, name="warm_a", tag="warm")
        nc.gpsimd.memset(warm_a[:], 0.0)
        def junk_mms(n, nw=128):
            for wi in range(n):
                wac = psp.tile([128, nw], F32, name="wac", tag="qkv",
                               padded_shape=[128, SBW])
                nc.tensor.matmul(wac[:], warm_a[:, 0:128],
                                 warm_a[:, 0:nw], start=True, stop=True)

        junk_mms(NWARM)

        # ---- input DMAs, by time of first use ----
        # scalar ring (ACT; idle at the head): consts, wq0, wk0, wq1, wk1
        # sync ring: xt0a, xt0b, wv, xt1, wp, xt2, xt3
        bqk = cst.tile([128, 4], F32, name="bqk", tag="bqk")
        nc.scalar.dma_start(bqk[:], d["bqk"][:])
        wq = big.tile([128, WQW], BF16, name="wq", tag="wq")
        nc.scalar.dma_start(wq[:, 0:1024], d["wq"][:, 0:1024])
        wk = big.tile([128, NEB * EL], BF16, name="wk", tag="wk")
        nc.scalar.dma_start(wk[:, 0:1024], d["wk"][:, 0:1024])
        bq = bqk[:, 0:2]
        bk = bqk[:, 2:4]
        bv = wq[:, PBV:PBV + EL]
        tri2 = wq[:, PUT:PUT + 256]

        xts = [big.tile([128, NEB * SBW], BF16, name=f"xt{i}", tag=f"xt{i}")
               for i in range(NSB)]
        # balance the critical bytes: sync gets 6 e-blocks of xt0, scalar
        # gets the last 2 (after wq0/wk0); later weights follow
        c6 = 6 * SBW
        nc.sync.dma_start(xts[0][:, 0:c6], d["xt"][:, 0:c6])
        nc.scalar.dma_start(xts[0][:, c6:NEB * SBW], d["xt"][:, c6:NEB * SBW])
        nc.scalar.dma_start(wq[:, 1024:2048], d["wq"][:, 1024:2048])
        nc.scalar.dma_start(wk[:, 1024:2048], d["wk"][:, 1024:2048])
        nc.scalar.dma_start(wq[:, WQP:WQW], d["wq"][:, WQP:WQW])
        wv = big.tile([128, NEB * EL], BF16, name="wv", tag="wv")
        nc.sync.dma_start(wv[:], d["wv"][:])
        nc.sync.dma_start(xts[1][:], d["xt"][:, NEB * SBW: 2 * NEB * SBW])
        wp = big.tile([128, 2 * E], BF16, name="wp", tag="wp")
        nc.sync.dma_start(wp[:], d["wp"][:])
        for i in (2, 3):
            nc.sync.dma_start(xts[i][:],
                              d["xt"][:, i * NEB * SBW:(i + 1) * NEB * SBW])

        junk_mms(NWARM2, 512)

        # V tiles [128, 4*128]: head h = [ones (64) | v_h (64)] at 128h, so
        # the AV matmul puts softmax denominators (replicated 64x) in PSUM
        # rows 0:64 and values in rows 64:128
        vt = []
        for j in range(NTB):
            t = big.tile([128, HPC * 128], BF16, name=f"vt{j}", tag=f"vt{j}")
            nc.gpsimd.memset(
                t.rearrange("p (h c) -> p h c", c=128)[:, :, 0:64], 1.0
            )
            vt.append(t)

        qt = [big.tile([128, S], BF16, name=f"qt{k}", tag=f"qt{k}")
              for k in range(2)]
        kt = [big.tile([128, S], BF16, name=f"kt{k}", tag=f"kt{k}")
              for k in range(2)]
        yt = [big.tile([128, S], BF16, name=f"yt{k}", tag=f"yt{k}")
              for k in range(2)]

        def emit_proj_units(units, tail=False):
            for ui, (r0, nb2) in enumerate(units):
                pr = psp.tile([128, SBW], F32, name="pr", tag="qkv")
                for cb in range(2):
                    nc.tensor.matmul(
                        pr[:, 0:SBW],
                        yt[cb][:, r0:r0 + 128],
                        wp[:, cb * E + nb2 * 512: cb * E + (nb2 + 1) * 512],
                        start=(cb == 0),
                        stop=(cb == 1),
                    )
                ot = outp.tile([128, 512], BF16, name="ot", tag="ot")
                if tail and ui % 2 == 0:
                    nc.scalar.activation(ot[:], pr[:, 0:SBW], Copy)
                else:
                    nc.vector.tensor_copy(ot[:], pr[:, 0:SBW])
                ring = nc.scalar if (tail and ui % 2 == 1) else nc.sync
                ring.dma_start(
                    d["out"][r0:r0 + 128, nb2 * 512:(nb2 + 1) * 512], ot[:]
                )

        def norm_emit(avs_, p_, i_, nch, ch_hook=None):
            # normalize: yt rows = av[64:128] * 1/av[0:64], chunked
            # column-wise so downstream proj matmuls start early
            cw_n = SBW // nch
            bcrs = [bcsp.tile([64, SBW], F32, name="bcr", tag="bcs")
                    for _ in range(2)]
            for ch in range(nch):
                c0, c1 = ch * cw_n, (ch + 1) * cw_n
                for hh in range(2):
                    h = 2 * p_ + hh
                    dt_i, po = h // 2, 64 * (h % 2)
                    nc.vector.reciprocal_approx_fast(
                        bcrs[hh][:, c0:c1], avs_[hh][0:64, c0:c1])
                    nc.vector.tensor_mul(
                        yt[dt_i][po:po + 64,
                                 i_ * SBW + c0: i_ * SBW + c1],
                        avs_[hh][64:128, c0:c1],
                        bcrs[hh][:, c0:c1],
                    )
                if ch_hook is not None:
                    ch_hook(ch)

        def qkv_group_thunks(i):
            """One thunk per PSUM-tile group: q0,k0,q1,k1,v0..v3."""
            thunks = []
            for dst, wl, bl, dt_i in ((qt, wq, bq, 0), (kt, wk, bk, 0),
                                      (qt, wq, bq, 1), (kt, wk, bk, 1)):
                def th(dst=dst, wl=wl, bl=bl, dt_i=dt_i, i=i):
                    ac = psp.tile([128, SBW], F32, name="qk_ac", tag="qkv")
                    for j in range(NEB):
                        nc.tensor.matmul(
                            ac[:],
                            wl[:, dt_i * 1024 + j * 128:
                               dt_i * 1024 + j * 128 + 128],
                            xts[i][:, j * SBW:(j + 1) * SBW],
                            start=(j == 0),
                            stop=(j == NEB - 1),
                        )
                    nc.vector.tensor_scalar_add(
                        dst[dt_i][:, i * SBW:(i + 1) * SBW], ac[:],
                        bl[:, dt_i:dt_i + 1]
                    )
                thunks.append(th)
            for jt in range(4 * i, 4 * i + 4):
                def th(jt=jt, i=i):
                    ac = psp.tile([128, EL], F32, name="v_ac", tag="qkv",
                                  padded_shape=[128, SBW])
                    jl = jt * TBW - i * SBW
                    for eb in range(NEB):
                        nc.tensor.matmul(
                            ac[:],
                            xts[i][:, eb * SBW + jl: eb * SBW + jl + TBW],
                            wv[:, eb * EL:(eb + 1) * EL],
                            start=(eb == 0),
                            stop=(eb == NEB - 1),
                        )
                    nc.vector.tensor_add(
                        vt[jt].rearrange("p (h c) -> p h c",
                                         c=128)[:, :, 64:128],
                        ac.rearrange("p (h c) -> p h c", c=64),
                        bv.rearrange("p (h c) -> p h c", c=64),
                    )
                thunks.append(th)
            return thunks

        # block 0's QKV runs standalone (the warm-ups precede it)
        for th in qkv_group_thunks(0):
            th()

        proj_pending = []
        norm_pending = []
        for i in range(NSB):
            # previous block's deferred normalizes (single-chunk: consumers
            # are far away; fewer DVE instructions)
            for args in norm_pending:
                norm_emit(*args, 1)
            norm_pending = []

            # fillers dripped into this block's attention iterations:
            # next block's QKV groups first, then the previous block's
            # projection units
            # late blocks are exp-paced with PE slack, so projection
            # drips migrate toward them: block 2 keeps only half of
            # proj(1), block 3 absorbs the rest (real work displacing junk)
            ndrip = (len(proj_pending) if i != NSB - 2
                     else min(3, len(proj_pending)))
            drips, proj_pending = proj_pending[:ndrip], proj_pending[ndrip:]
            fillers = ([('qkv', th) for th in
                        (qkv_group_thunks(i + 1) if i < NSB - 1 else [])] +
                       [('proj', u) for u in drips])
            if i == NSB - 1:
                # no next-block QKV to keep the PE dense: drip junk matmuls
                # so the HAM clock gate stays open through the tail
                fillers = fillers + [('junk', None)] * 3
            njs = 4 * i + 4
            tot_iters = 2 * njs
            nf = len(fillers)
            fill_at = {}
            if nf:
                lo, hi = 2, max(3, tot_iters - 1)
                for fi in range(nf):
                    slot = min(hi, lo + (fi * (hi - lo)) // max(1, nf - 1)) \
                        if nf > 1 else lo
                    fill_at.setdefault(slot, []).append(fillers[fi])
            gj = 0
            for p in range(2):
                avs = [psp.tile([128, SBW], F32, name=f"av{p}{hh}",
                                tag="avs") for hh in range(2)]

                def av_mms(pt_, w_, j_, first_, last_):
                    for hh in range(2):
                        h = 2 * p + hh
                        nc.tensor.matmul(
                            avs[hh][:, w_:SBW],
                            vt[j_][:, 128 * h: 128 * h + 128],
                            pt_[:, hh * SBW + w_: (hh + 1) * SBW],
                            start=first_,
                            stop=last_,
                        )

                # diagonal t-blocks first: their post-exp DVE tri-mask gets
                # the whole wave to hide in, and the wave END flushes on
                # mask-free non-diag blocks
                js = list(range(4 * i, 4 * i + 4)) + list(range(0, 4 * i))
                pending = []  # AV deferred by two iterations
                for jx, j in enumerate(js):
                    w = 128 * (j - 4 * i) if j >= 4 * i else 0
                    cw = SBW - w
                    diag = j >= 4 * i
                    sc2 = psp.tile([128, 2 * SBW], F32, name="sc2",
                                   tag="scpr")
                    for hh in range(2):
                        h = 2 * p + hh
                        dt_i, po = h // 2, 64 * (h % 2)
                        nc.tensor.matmul(
                            sc2[:, hh * SBW: hh * SBW + cw],
                            kt[dt_i][po:po + 64, j * TBW:(j + 1) * TBW],
                            qt[dt_i][po:po + 64, i * SBW + w:(i + 1) * SBW],
                            start=True,
                            stop=True,
                        )
                    pt_t = ptp.tile([128, 2 * SBW], BF16, name="ptile",
                                    tag="pt")
                    nc.scalar.activation(
                        pt_t.rearrange("q (g c) -> q g c", c=SBW)[:, :, w:SBW],
                        sc2.rearrange("q (g c) -> q g c", c=SBW)[:, :, 0:cw],
                        Exp,
                    )
                    if diag:  # zero above the causal diagonal: bf16
                        # SBUF-only multiply by the upper-tri keep-mask
                        # (4x-mode DVE, both head groups in one op)
                        nc.vector.tensor_mul(
                            pt_t.rearrange("q (g c) -> q g c",
                                           c=SBW)[:, :, w:w + 128],
                            pt_t.rearrange("q (g c) -> q g c",
                                           c=SBW)[:, :, w:w + 128],
                            tri2.rearrange("q (g c) -> q g c", c=128),
                        )
                    pending.append((pt_t, w, j, jx == 0, jx == njs - 1))
                    if len(pending) > 3:
                        av_mms(*pending.pop(0))
                    for kind, payload in fill_at.pop(gj, []):
                        if kind == 'qkv':
                            payload()
                        elif kind == 'junk':
                            junk_mms(2, 256)
                        else:
                            emit_proj_units([payload])
                    gj += 1
                for it in pending:
                    av_mms(*it)
                if i == NSB - 1:
                    if p == 0:
                        norm_emit(avs, p, i, 1)
                    else:
                        # keep the PE (and the HAM clock) warm through the
                        # normalize window, then interleave the final
                        # normalize with its projection units
                        junk_mms(10, 256)
                        units = [[(i * SBW + st * 128, nb2)
                                  for nb2 in range(2)] for st in range(4)]
                        norm_emit(avs, p, i, 4,
                                  ch_hook=lambda ch: emit_proj_units(
                                      units[ch], tail=True))
                else:
                    norm_pending.append((avs, p, i))
            # any fillers not consumed (shouldn't happen) flush here
            for slot in sorted(fill_at):
                for kind, payload in fill_at[slot]:
                    if kind == 'qkv':
                        payload()
                    else:
                        emit_proj_units([payload])
            fill_at = {}
            if i < NSB - 1:
                proj_pending = proj_pending + [
                    (i * SBW + st * 128, nb2)
                    for st in range(4) for nb2 in range(2)]


def _build():
    global _BUILT
    if _BUILT is not None:
        return _BUILT
    nc = bacc.Bacc("TRN2", target_bir_lowering=False, debug=False,
                   num_devices=NCORES)
    d = {
        "xt": nc.dram_tensor("xt", [128, NSB * NEB * SBW], BF16, kind="ExternalInput").ap(),
        "wq": nc.dram_tensor("wq", [128, WQW], BF16, kind="ExternalInput").ap(),
        "wk": nc.dram_tensor("wk", [128, NEB * EL], BF16, kind="ExternalInput").ap(),
        "wv": nc.dram_tensor("wv", [128, NEB * EL], BF16, kind="ExternalInput").ap(),
        "wp": nc.dram_tensor("wp", [128, 2 * E], BF16, kind="ExternalInput").ap(),
        "bqk": nc.dram_tensor("bqk", [128, 4], F32, kind="ExternalInput").ap(),
        "out": nc.dram_tensor("out", [S, E], BF16, kind="ExternalOutput").ap(),
    }
    with tile.TileContext(nc) as tc:
        _emit(tc, nc, d)
    nc.compile()
    _BUILT = nc
    return _BUILT


def _blockify(a, pblk):
    """[N*pblk, M] -> [pblk, N*M] with block-column layout."""
    n = a.shape[0] // pblk
    return np.ascontiguousarray(
        a.reshape(n, pblk, a.shape[1]).transpose(1, 0, 2).reshape(pblk, -1)
    )


def _blockify_dt(a, pblk):
    """[N*pblk, 2*128] -> [pblk, 2 * N*128], dt-major block-column layout."""
    n = a.shape[0] // pblk
    # [n, pblk, 2, 128] -> [pblk, 2, n, 128]
    return np.ascontiguousarray(
        a.reshape(n, pblk, 2, 128).transpose(1, 2, 0, 3).reshape(pblk, -1)
    )


def _prep_core(c, x, Wq, bq, Wk, bk, Wv, bv, Wp):
    b, g = c // 4, c % 4
    lo = EL * g
    bf = ml_dtypes.bfloat16

    xT = np.ascontiguousarray(x[b].T)                        # [E, S]
    wqT = np.ascontiguousarray(Wq[lo:lo + EL, :].T) * SCALE  # [E, 256]
    wkT = np.ascontiguousarray(Wk[lo:lo + EL, :].T)
    wvT = np.ascontiguousarray(Wv[lo:lo + EL, :].T)
    wpT = np.ascontiguousarray(Wp[:, lo:lo + EL].T)          # [256, E]

    # xt: s-block-major consolidated layout: col (i*NEB + j)*SBW + s
    # holds xT[j*128 + p, i*SBW + s]
    xt = (xT.reshape(NEB, 128, NSB, SBW).transpose(1, 2, 0, 3)
          .reshape(128, NSB * NEB * SBW))

    col = np.arange(128)
    # tri[t, c] = 1 iff c >= t: upper-tri (incl diag) keep-mask for the
    # causal boundary, stored twice side-by-side (one copy per head group)
    tri = (col[None, :] >= col[:, None]).astype(np.float64)

    wq_pack = np.concatenate([
        _blockify_dt(wqT, 128),
        np.ascontiguousarray(np.broadcast_to(bv[lo:lo + EL], (128, EL))),
        tri,
        tri,
    ], axis=1)
    bqk = np.concatenate([
        np.ascontiguousarray((bq[lo:lo + EL] * SCALE).reshape(2, 128).T),
        np.ascontiguousarray(bk[lo:lo + EL].reshape(2, 128).T),
    ], axis=1).astype(np.float32)

    return {
        "xt": np.ascontiguousarray(xt).astype(bf),
        "wq": np.ascontiguousarray(wq_pack).astype(bf),
        "wk": _blockify_dt(wkT, 128).astype(bf),
        "wv": _blockify(wvT, 128).astype(bf),
        "wp": _blockify(wpT, 128).astype(bf),
        "bqk": bqk,
    }


def run(inputs, trace=False):
    """Run on hardware. Returns (out [B,S,E] f32, exec_time_ns or None)."""
    x = np.asarray(inputs["x"], np.float32)
    Wq = np.asarray(inputs["Wq"], np.float32)
    bq = np.asarray(inputs["bq"], np.float32)
    Wk = np.asarray(inputs["Wk"], np.float32)
    bk = np.asarray(inputs["bk"], np.float32)
    Wv = np.asarray(inputs["Wv"], np.float32)
    bv = np.asarray(inputs["bv"], np.float32)
    Wp = np.asarray(inputs["Wp"], np.float32)
    bp = np.asarray(inputs["bp"], np.float32)

    nc = _build()
    in_maps = [
        _prep_core(c, x, Wq, bq, Wk, bk, Wv, bv, Wp) for c in range(NCORES)
    ]
    kwargs = {}
    if trace:
        try:
            import ntff_shim
            ntff_shim.install()
        except Exception:
            pass
        kwargs["trace"] = True
        kwargs["tmpdir"] = "/tmp/trace_out"
        import os
        import shutil
        shutil.rmtree("/tmp/trace_out", ignore_errors=True)
        os.makedirs("/tmp/trace_out", exist_ok=True)
    res = bass_utils.run_bass_kernel_spmd(
        nc, in_maps, list(range(NCORES)), **kwargs
    )
    global LAST_RESULT
    LAST_RESULT = res
    out = np.empty((B, S, E), np.float32)
    for b in range(B):
        acc = res.results[4 * b]["out"].astype(np.float32)
        for g in range(1, 4):
            acc = acc + res.results[4 * b + g]["out"].astype(np.float32)
        out[b] = acc + bp[None, :]
    return out, res.exec_time_ns


def kernel(**inputs):
    out, _ = run(inputs, trace=False)
    return out


# revision 22
# speedup vs baseline: 1.0028x; 1.0028x over previous
"""Multi-head causal self-attention (B=2, S=2048, E=1024, H=16, D=64) on 8 TRN2
NeuronCores.

Sharding: core c owns batch b = c//4 and head-group g = c%4 (4 heads each).
Per core, transpose-free layout: QT/KT [d_local=256, S] (d on partitions),
V [S, d_local] (t on partitions), scoresT [t, s] blocks via lhsT=KT-block,
softmax is unnormalized exp (scores ~N(0,1) in f32), output projection is
row-parallel (each core computes a partial [S, E] with its 256 channels of
Wp; host sums 4 bf16 partials per batch in f32 and adds bp).

Schedule: QKV projection for s-block i is interleaved with attention for
s-block i-1..i (causal: attention block i only needs K/V t-blocks <= 4i+3),
so the ACT-heavy attention exps overlap the PE-dense projection matmuls.

Softmax denominators come free from V tiles augmented per head with 64 ones
columns ([1s | v_h] -> M=128 AV matmul; rows 0:64 of the AV PSUM tile hold
the column sums replicated 64x), so normalization is a full-lane DVE
reciprocal + multiply. The causal mask is a post-exp DVE multiply of the
diagonal P block by an upper-tri 0/1 bf16 mask (SBUF-only 4x-mode, both
head groups in one op); waves process their diagonal t-blocks FIRST so
the mask hop hides inside the wave and the wave end flushes on mask-free
blocks.

Head pairs run as two sequential waves per s-block (PSUM economy); heads
within a pair sit at partition bases 0/64 so their K=64 score matmuls
run at the PE's double-rate half-contraction mode. AV consumes PT three
iterations later. Each block's attention iterations carry fillers: the
NEXT block's QKV PSUM-tile groups and earlier blocks' projection units,
weighted toward the exp-paced late blocks (block 2 keeps half of
proj(1); block 3 absorbs the rest), so the PE never drains while the
ACT engine works through the softmax exps.

Head pipelining: weights are dt-major so wq/wk split into per-dt halves;
the first x s-block splits into two 4-e-block halves. DMA order is
by time-of-first-use (scalar ring: biases+mask consts, wq0, wk0, wq1, wk1;
sync ring: xt0a, xt0b, wv, xt1, wp, xt2, xt3), so the first QKV tile's
inputs land ~4us earlier than a monolithic load. Junk warm-up matmuls
bridge the preamble->data window to keep the HAM clock gate open. The
final block's p=1 normalize is chunk-interleaved with its projection
units, whose PSUM drains alternate ACT/DVE and whose output DMAs
alternate sync/scalar rings.

Weights/activations are bf16 (matmul inputs); accumulation f32 in PSUM;
softmax exp/normalization f32; partial outputs bf16.
"""

import numpy as np
import ml_dtypes

import concourse.bass as bass
import concourse.tile as tile
from concourse import bacc, mybir
from concourse import bass_utils

B, S, E, H, D = 2, 2048, 1024, 16, 64
NCORES = 8
HPC = 4                 # heads per core
EL = HPC * D            # 256 local channels
SBW = 512               # s-block width
NSB = S // SBW          # 4
TBW = 128               # t-block width
NTB = S // TBW          # 16
NEB = E // 128          # 8 e-blocks
SCALE = 1.0 / np.sqrt(D)
NWARM = 8
NWARM2 = 14

F32 = mybir.dt.float32
BF16 = mybir.dt.bfloat16

# wq pack layout (bf16): [wq dt-major 2048 | bv 256 | tri 128 | tri 128]
WQP = NEB * EL
PBV = WQP          # bv
PUT = WQP + EL
WQW = PUT + 256

_BUILT = None


def _emit(tc, nc, d):
    Exp = mybir.ActivationFunctionType.Exp
    Copy = mybir.ActivationFunctionType.Copy

    with (
        tc.tile_pool(name="const", bufs=1) as cst,
        tc.tile_pool(name="big", bufs=1) as big,
        tc.tile_pool(name="ptp", bufs=8) as ptp,
        tc.tile_pool(name="bcsp", bufs=2) as bcsp,
        tc.tile_pool(name="outp", bufs=4) as outp,
        tc.tile_pool(name="ps", bufs=2, space="PSUM") as psp,
    ):
        # ---- PE warm-up: junk matmuls from preamble end (gated only by a
        # fast gpsimd memset) keep the PE busy through the input DMA window
        # so the HAM clock gate unthrottles early ----
        warm_a = cst.tile([128, 512], BF16, name="warm_a", tag="warm")
        nc.gpsimd.memset(warm_a[:], 0.0)
        def junk_mms(n, nw=128):
            for wi in range(n):
                wac = psp.tile([128, nw], F32, name="wac", tag="qkv",
                               padded_shape=[128, SBW])
                nc.tensor.matmul(wac[:], warm_a[:, 0:128],
                                 warm_a[:, 0:nw], start=True, stop=True)

        junk_mms(NWARM)

        # ---- input DMAs, by time of first use ----
        # scalar ring (ACT; idle at the head): consts, wq0, wk0, wq1, wk1
        # sync ring: xt0a, xt0b, wv, xt1, wp, xt2, xt3
        bqk = cst.tile([128, 4], F32, name="bqk", tag="bqk")
        nc.scalar.dma_start(bqk[:], d["bqk"][:])
        wq = big.tile([128, WQW], BF16, name="wq", tag="wq")
        nc.scalar.dma_start(wq[:, 0:1024], d["wq"][:, 0:1024])
        wk = big.tile([128, NEB * EL], BF16, name="wk", tag="wk")
        nc.scalar.dma_start(wk[:, 0:1024], d["wk"][:, 0:1024])
        bq = bqk[:, 0:2]
        bk = bqk[:, 2:4]
        bv = wq[:, PBV:PBV + EL]
        tri2 = wq[:, PUT:PUT + 256]

        xts = [big.tile([128, NEB * SBW], BF16, name=f"xt{i}", tag=f"xt{i}")
               for i in range(NSB)]
        # balance the critical bytes: sync gets 6 e-blocks of xt0, scalar
        # gets the last 2 (after wq0/wk0); later weights follow
        c6 = 6 * SBW
        nc.sync.dma_start(xts[0][:, 0:c6], d["xt"][:, 0:c6])
        nc.scalar.dma_start(xts[0][:, c6:NEB * SBW], d["xt"][:, c6:NEB * SBW])
        nc.scalar.dma_start(wq[:, 1024:2048], d["wq"][:, 1024:2048])
        nc.scalar.dma_start(wk[:, 1024:2048], d["wk"][:, 1024:2048])
        nc.scalar.dma_start(wq[:, WQP:WQW], d["wq"][:, WQP:WQW])
        wv = big.tile([128, NEB * EL], BF16, name="wv", tag="wv")
        nc.sync.dma_start(wv[:], d["wv"][:])
        nc.sync.dma_start(xts[1][:], d["xt"][:, NEB * SBW: 2 * NEB * SBW])
        wp = big.tile([128, 2 * E], BF16, name="wp", tag="wp")
        nc.sync.dma_start(wp[:], d["wp"][:])
        for i in (2, 3):
            nc.sync.dma_start(xts[i][:],
                              d["xt"][:, i * NEB * SBW:(i + 1) * NEB * SBW])

        junk_mms(NWARM2, 512)

        # V tiles [128, 4*128]: head h = [ones (64) | v_h (64)] at 128h, so
        # the AV matmul puts softmax denominators (replicated 64x) in PSUM
        # rows 0:64 and values in rows 64:128
        vt = []
        for j in range(NTB):
            t = big.tile([128, HPC * 128], BF16, name=f"vt{j}", tag=f"vt{j}")
            nc.gpsimd.memset(
                t.rearrange("p (h c) -> p h c", c=128)[:, :, 0:64], 1.0
            )
            vt.append(t)

        qt = [big.tile([128, S], BF16, name=f"qt{k}", tag=f"qt{k}")
              for k in range(2)]
        kt = [big.tile([128, S], BF16, name=f"kt{k}", tag=f"kt{k}")
              for k in range(2)]
        yt = [big.tile([128, S], BF16, name=f"yt{k}", tag=f"yt{k}")
              for k in range(2)]

        def emit_proj_units(units, tail=False):
            for ui, (r0, nb2) in enumerate(units):
                pr = psp.tile([128, SBW], F32, name="pr", tag="qkv")
                for cb in range(2):
                    nc.tensor.matmul(
                        pr[:, 0:SBW],
                        yt[cb][:, r0:r0 + 128],
                        wp[:, cb * E + nb2 * 512: cb * E + (nb2 + 1) * 512],
                        start=(cb == 0),
                        stop=(cb == 1),
                    )
                ot = outp.tile([128, 512], BF16, name="ot", tag="ot")
                if tail and ui % 2 == 0:
                    nc.scalar.activation(ot[:], pr[:, 0:SBW], Copy)
                else:
                    nc.vector.tensor_copy(ot[:], pr[:, 0:SBW])
                ring = nc.scalar if (tail and ui % 2 == 1) else nc.sync
                ring.dma_start(
                    d["out"][r0:r0 + 128, nb2 * 512:(nb2 + 1) * 512], ot[:]
                )

        def norm_emit(avs_, p_, i_, nch, ch_hook=None):
            # normalize: yt rows = av[64:128] * 1/av[0:64], chunked
            # column-wise so downstream proj matmuls start early
            cw_n = SBW // nch
            bcrs = [bcsp.tile([64, SBW], F32, name="bcr", tag="bcs")
                    for _ in range(2)]
            for ch in range(nch):
                c0, c1 = ch * cw_n, (ch + 1) * cw_n
                for hh in range(2):
                    h = 2 * p_ + hh
                    dt_i, po = h // 2, 64 * (h % 2)
                    nc.vector.reciprocal_approx_fast(
                        bcrs[hh][:, c0:c1], avs_[hh][0:64, c0:c1])
                    nc.vector.tensor_mul(
                        yt[dt_i][po:po + 64,
                                 i_ * SBW + c0: i_ * SBW + c1],
                        avs_[hh][64:128, c0:c1],
                        bcrs[hh][:, c0:c1],
                    )
                if ch_hook is not None:
                    ch_hook(ch)

        def qkv_group_thunks(i):
            """One thunk per PSUM-tile group: q0,k0,q1,k1,v0..v3."""
            thunks = []
            for dst, wl, bl, dt_i in ((qt, wq, bq, 0), (kt, wk, bk, 0),
                                      (qt, wq, bq, 1), (kt, wk, bk, 1)):
                def th(dst=dst, wl=wl, bl=bl, dt_i=dt_i, i=i):
                    ac = psp.tile([128, SBW], F32, name="qk_ac", tag="qkv")
                    for j in range(NEB):
                        nc.tensor.matmul(
                            ac[:],
                            wl[:, dt_i * 1024 + j * 128:
                               dt_i * 1024 + j * 128 + 128],
                            xts[i][:, j * SBW:(j + 1) * SBW],
                            start=(j == 0),
                            stop=(j == NEB - 1),
                        )
                    nc.vector.tensor_scalar_add(
                        dst[dt_i][:, i * SBW:(i + 1) * SBW], ac[:],
                        bl[:, dt_i:dt_i + 1]
                    )
                thunks.append(th)
            for jt in range(4 * i, 4 * i + 4):
                def th(jt=jt, i=i):
                    ac = psp.tile([128, EL], F32, name="v_ac", tag="qkv",
                                  padded_shape=[128, SBW])
                    jl = jt * TBW - i * SBW
                    for eb in range(NEB):
                        nc.tensor.matmul(
                            ac[:],
                            xts[i][:, eb * SBW + jl: eb * SBW + jl + TBW],
                            wv[:, eb * EL:(eb + 1) * EL],
                            start=(eb == 0),
                            stop=(eb == NEB - 1),
                        )
                    nc.vector.tensor_add(
                        vt[jt].rearrange("p (h c) -> p h c",
                                         c=128)[:, :, 64:128],
                        ac.rearrange("p (h c) -> p h c", c=64),
                        bv.rearrange("p (h c) -> p h c", c=64),
                    )
                thunks.append(th)
            return thunks

        # block 0's QKV runs standalone (the warm-ups precede it)
        for th in qkv_group_thunks(0):
            th()

        proj_pending = []
        norm_pending = []
        for i in range(NSB):
            # previous block's deferred normalizes (single-chunk: consumers
            # are far away; fewer DVE instructions)
            for args in norm_pending:
                norm_emit(*args, 1)
            norm_pending = []

            # fillers dripped into this block's attention iterations:
            # next block's QKV groups first, then the previous block's
            # projection units
            # late blocks are exp-paced with PE slack, so projection
            # drips migrate toward them: block 2 keeps only half of
            # proj(1), block 3 absorbs the rest (real work displacing junk)
            ndrip = (len(proj_pending) if i != NSB - 2
                     else min(4, len(proj_pending)))
            drips, proj_pending = proj_pending[:ndrip], proj_pending[ndrip:]
            fillers = ([('qkv', th) for th in
                        (qkv_group_thunks(i + 1) if i < NSB - 1 else [])] +
                       [('proj', u) for u in drips])
            if i == NSB - 1:
                # no next-block QKV to keep the PE dense: drip junk matmuls
                # so the HAM clock gate stays open through the tail
                fillers = fillers + [('junk', None)] * 3
            njs = 4 * i + 4
            tot_iters = 2 * njs
            nf = len(fillers)
            fill_at = {}
            if nf:
                lo, hi = 2, max(3, tot_iters - 1)
                for fi in range(nf):
                    slot = min(hi, lo + (fi * (hi - lo)) // max(1, nf - 1)) \
                        if nf > 1 else lo
                    fill_at.setdefault(slot, []).append(fillers[fi])
            gj = 0
            for p in range(2):
                avs = [psp.tile([128, SBW], F32, name=f"av{p}{hh}",
                                tag="avs") for hh in range(2)]

                def av_mms(pt_, w_, j_, first_, last_):
                    for hh in range(2):
                        h = 2 * p + hh
                        nc.tensor.matmul(
                            avs[hh][:, w_:SBW],
                            vt[j_][:, 128 * h: 128 * h + 128],
                            pt_[:, hh * SBW + w_: (hh + 1) * SBW],
                            start=first_,
                            stop=last_,
                        )

                # diagonal t-blocks first: their post-exp DVE tri-mask gets
                # the whole wave to hide in, and the wave END flushes on
                # mask-free non-diag blocks
                js = list(range(4 * i, 4 * i + 4)) + list(range(0, 4 * i))
                pending = []  # AV deferred by two iterations
                for jx, j in enumerate(js):
                    w = 128 * (j - 4 * i) if j >= 4 * i else 0
                    cw = SBW - w
                    diag = j >= 4 * i
                    sc2 = psp.tile([128, 2 * SBW], F32, name="sc2",
                                   tag="scpr")
                    for hh in range(2):
                        h = 2 * p + hh
                        dt_i, po = h // 2, 64 * (h % 2)
                        nc.tensor.matmul(
                            sc2[:, hh * SBW: hh * SBW + cw],
                            kt[dt_i][po:po + 64, j * TBW:(j + 1) * TBW],
                            qt[dt_i][po:po + 64, i * SBW + w:(i + 1) * SBW],
                            start=True,
                            stop=True,
                        )
                    pt_t = ptp.tile([128, 2 * SBW], BF16, name="ptile",
                                    tag="pt")
                    nc.scalar.activation(
                        pt_t.rearrange("q (g c) -> q g c", c=SBW)[:, :, w:SBW],
                        sc2.rearrange("q (g c) -> q g c", c=SBW)[:, :, 0:cw],
                        Exp,
                    )
                    if diag:  # zero above the causal diagonal: bf16
                        # SBUF-only multiply by the upper-tri keep-mask
                        # (4x-mode DVE, both head groups in one op)
                        nc.vector.tensor_mul(
                            pt_t.rearrange("q (g c) -> q g c",
                                           c=SBW)[:, :, w:w + 128],
                            pt_t.rearrange("q (g c) -> q g c",
                                           c=SBW)[:, :, w:w + 128],
                            tri2.rearrange("q (g c) -> q g c", c=128),
                        )
                    pending.append((pt_t, w, j, jx == 0, jx == njs - 1))
                    if len(pending) > 3:
                        av_mms(*pending.pop(0))
                    for kind, payload in fill_at.pop(gj, []):
                        if kind == 'qkv':
                            payload()
                        elif kind == 'junk':
                            junk_mms(2, 256)
                        else:
                            emit_proj_units([payload])
                    gj += 1
                for it in pending:
                    av_mms(*it)
                if i == NSB - 1:
                    if p == 0:
                        norm_emit(avs, p, i, 1)
                    else:
                        # keep the PE (and the HAM clock) warm through the
                        # normalize window, then interleave the final
                        # normalize with its projection units
                        junk_mms(10, 256)
                        units = [[(i * SBW + st * 128, nb2)
                                  for nb2 in range(2)] for st in range(4)]
                        norm_emit(avs, p, i, 4,
                                  ch_hook=lambda ch: emit_proj_units(
                                      units[ch], tail=True))
                else:
                    norm_pending.append((avs, p, i))
            # any fillers not consumed (shouldn't happen) flush here
            for slot in sorted(fill_at):
                for kind, payload in fill_at[slot]:
                    if kind == 'qkv':
                        payload()
                    else:
                        emit_proj_units([payload])
            fill_at = {}
            if i < NSB - 1:
                proj_pending = proj_pending + [
                    (i * SBW + st * 128, nb2)
                    for st in range(4) for nb2 in range(2)]


def _build():
    global _BUILT
    if _BUILT is not None:
        return _BUILT
    nc = bacc.Bacc("TRN2", target_bir_lowering=False, debug=False,
                   num_devices=NCORES)
    d = {
        "xt": nc.dram_tensor("xt", [128, NSB * NEB * SBW], BF16, kind="ExternalInput").ap(),
        "wq": nc.dram_tensor("wq", [128, WQW], BF16, kind="ExternalInput").ap(),
        "wk": nc.dram_tensor("wk", [128, NEB * EL], BF16, kind="ExternalInput").ap(),
        "wv": nc.dram_tensor("wv", [128, NEB * EL], BF16, kind="ExternalInput").ap(),
        "wp": nc.dram_tensor("wp", [128, 2 * E], BF16, kind="ExternalInput").ap(),
        "bqk": nc.dram_tensor("bqk", [128, 4], F32, kind="ExternalInput").ap(),
        "out": nc.dram_tensor("out", [S, E], BF16, kind="ExternalOutput").ap(),
    }
    with tile.TileContext(nc) as tc:
        _emit(tc, nc, d)
    nc.compile()
    _BUILT = nc
    return _BUILT


def _blockify(a, pblk):
    """[N*pblk, M] -> [pblk, N*M] with block-column layout."""
    n = a.shape[0] // pblk
    return np.ascontiguousarray(
        a.reshape(n, pblk, a.shape[1]).transpose(1, 0, 2).reshape(pblk, -1)
    )


def _blockify_dt(a, pblk):
    """[N*pblk, 2*128] -> [pblk, 2 * N*128], dt-major block-column layout."""
    n = a.shape[0] // pblk
    # [n, pblk, 2, 128] -> [pblk, 2, n, 128]
    return np.ascontiguousarray(
        a.reshape(n, pblk, 2, 128).transpose(1, 2, 0, 3).reshape(pblk, -1)
    )


def _prep_core(c, x, Wq, bq, Wk, bk, Wv, bv, Wp):
    b, g = c // 4, c % 4
    lo = EL * g
    bf = ml_dtypes.bfloat16

    xT = np.ascontiguousarray(x[b].T)                        # [E, S]
    wqT = np.ascontiguousarray(Wq[lo:lo + EL, :].T) * SCALE  # [E, 256]
    wkT = np.ascontiguousarray(Wk[lo:lo + EL, :].T)
    wvT = np.ascontiguousarray(Wv[lo:lo + EL, :].T)
    wpT = np.ascontiguousarray(Wp[:, lo:lo + EL].T)          # [256, E]

    # xt: s-block-major consolidated layout: col (i*NEB + j)*SBW + s
    # holds xT[j*128 + p, i*SBW + s]
    xt = (xT.reshape(NEB, 128, NSB, SBW).transpose(1, 2, 0, 3)
          .reshape(128, NSB * NEB * SBW))

    col = np.arange(128)
    # tri[t, c] = 1 iff c >= t: upper-tri (incl diag) keep-mask for the
    # causal boundary, stored twice side-by-side (one copy per head group)
    tri = (col[None, :] >= col[:, None]).astype(np.float64)

    wq_pack = np.concatenate([
        _blockify_dt(wqT, 128),
        np.ascontiguousarray(np.broadcast_to(bv[lo:lo + EL], (128, EL))),
        tri,
        tri,
    ], axis=1)
    bqk = np.concatenate([
        np.ascontiguousarray((bq[lo:lo + EL] * SCALE).reshape(2, 128).T),
        np.ascontiguousarray(bk[lo:lo + EL].reshape(2, 128).T),
    ], axis=1).astype(np.float32)

    return {
        "xt": np.ascontiguousarray(xt).astype(bf),
        "wq": np.ascontiguousarray(wq_pack).astype(bf),
        "wk": _blockify_dt(wkT, 128).astype(bf),
        "wv": _blockify(wvT, 128).astype(bf),
        "wp": _blockify(wpT, 128).astype(bf),
        "bqk": bqk,
    }


def run(inputs, trace=False):
    """Run on hardware. Returns (out [B,S,E] f32, exec_time_ns or None)."""
    x = np.asarray(inputs["x"], np.float32)
    Wq = np.asarray(inputs["Wq"], np.float32)
    bq = np.asarray(inputs["bq"], np.float32)
    Wk = np.asarray(inputs["Wk"], np.float32)
    bk = np.asarray(inputs["bk"], np.float32)
    Wv = np.asarray(inputs["Wv"], np.float32)
    bv = np.asarray(inputs["bv"], np.float32)
    Wp = np.asarray(inputs["Wp"], np.float32)
    bp = np.asarray(inputs["bp"], np.float32)

    nc = _build()
    in_maps = [
        _prep_core(c, x, Wq, bq, Wk, bk, Wv, bv, Wp) for c in range(NCORES)
    ]
    kwargs = {}
    if trace:
        try:
            import ntff_shim
            ntff_shim.install()
        except Exception:
            pass
        kwargs["trace"] = True
        kwargs["tmpdir"] = "/tmp/trace_out"
        import os
        import shutil
        shutil.rmtree("/tmp/trace_out", ignore_errors=True)
        os.makedirs("/tmp/trace_out", exist_ok=True)
    res = bass_utils.run_bass_kernel_spmd(
        nc, in_maps, list(range(NCORES)), **kwargs
    )
    global LAST_RESULT
    LAST_RESULT = res
    out = np.empty((B, S, E), np.float32)
    for b in range(B):
        acc = res.results[4 * b]["out"].astype(np.float32)
        for g in range(1, 4):
            acc = acc + res.results[4 * b + g]["out"].astype(np.float32)
        out[b] = acc + bp[None, :]
    return out, res.exec_time_ns


def kernel(**inputs):
    out, _ = run(inputs, trace=False)
    return out
